# revision 13
# baseline (speedup 1.0000x reference)
"""Two-layer GCN (PyG GCNConv x2 + ReLU) on 8 Trainium2 NeuronCores.

Strategy (dst-sharded SPMD, v2):
  - Nodes padded to 102400, dealt degree-balanced over 800 (core, block)
    positions; 12800 dst rows per core, grouped as 25 quads of 512 (one
    PSUM bank each), super-quads of 5 quads.
  - Layer 1 dense (x @ W1) is REPLICATED on every core (PE is cheap, the
    f32 AllGather it replaces is not): each core writes a private full
    bf16 table [102400, 128]; edges gather from it in 4 windows of 25600
    rows (int16 indices).
  - Scatter-add runs on the tensor engine: per 128-edge tile a one-hot
    selection matrix S[e, dst_slot] = norm_e is built ON DEVICE by one DVE
    tensor_scalar (iota == dst) * norm from a tiny f32 sidecar, then
    matmul(lhsT=msg, rhs=S) accumulates into the quad's PSUM bank across
    all windows; ReLU+bias is fused into the single PSUM->SBUF epilogue.
  - Layer 2 dense (relu1 @ W2) is sharded; its bf16 AllGather is split
    into 4 chunk collectives issued as soon as the contributing quads
    finish, overlapping layer-1 edge tail; layer-2 edges whose source is
    core-local gather from the local bounce before any chunk lands.
  - Both edge phases run "transposed" (psum[feat, dst]); the final output
    is written [feat, 12800] per core and untransposed/unpermuted on host.
"""

import numpy as np
import ml_dtypes

import concourse.bass as bass
import concourse.bacc as bacc
import concourse.mybir as mybir
import concourse.tile as tile
from concourse.bass_utils import run_bass_kernel_spmd

N = 100000
E = 640000
D = 128
NCORES = 8
NPAD = 102400
SHARD = NPAD // NCORES        # 12800
QUAD = 512                    # dst rows per PSUM bank
NQ = SHARD // QUAD            # 25 quads per core
SQ = 5                        # quads per super-quad
NSQ = NQ // SQ                # 5 super-quads
WIN = 25600                   # L1 gather window rows (int16-safe)
NW1 = NPAD // WIN             # 4 windows (layer 1)
AGCH = SHARD // 4             # 3200 rows per AllGather chunk (layer 2)
CHUNK_T = 8                   # max tiles (of 128 edges) per dma_gather call

_CACHE = {}
import os
K_DEBUG = bool(int(os.environ.get("K_DEBUG", "0")))


def _pack_idx(gidx, t0, nt):
    """Wrapped int16 index layout for one dma_gather call."""
    blk = gidx[t0 * 128:(t0 + nt) * 128].reshape(nt * 8, 16).T
    return np.tile(blk, (8, 1))


def _host_prep(x, edge_index, W1, b1, W2, b2):
    x = np.asarray(x, dtype=np.float32)
    ei = np.asarray(edge_index)
    W1 = np.asarray(W1, dtype=np.float32)
    W2 = np.asarray(W2, dtype=np.float32)
    b1 = np.asarray(b1, dtype=np.float32)
    b2 = np.asarray(b2, dtype=np.float32)
    n = x.shape[0]

    src = np.concatenate([ei[0], np.arange(n, dtype=np.int64)])
    dst = np.concatenate([ei[1], np.arange(n, dtype=np.int64)])
    deg = np.bincount(dst, minlength=NPAD).astype(np.float32)
    a = np.zeros(NPAD, np.float32)
    nz = deg > 0
    a[nz] = 1.0 / np.sqrt(deg[nz])

    # degree-balanced node->position permutation (deal by degree over the
    # 800 (core, block) pairs), identical to the baseline kernel.
    NBLK = SHARD // 128
    order_by_deg = np.argsort(-deg, kind="stable")
    i = np.arange(NPAD, dtype=np.int64)
    cb = i % (NCORES * NBLK)
    position_of_rank = (cb % NCORES) * SHARD + (cb // NCORES) * 128 + i // (NCORES * NBLK)
    pos_of_node = np.empty(NPAD, np.int64)
    pos_of_node[order_by_deg] = position_of_rank
    node_at_pos = np.empty(NPAD, np.int64)
    node_at_pos[pos_of_node] = np.arange(NPAD, dtype=np.int64)

    ps = pos_of_node[src]
    pd = pos_of_node[dst]
    norm_e = a[src] * a[dst]
    core_e = pd // SHARD

    # per-layer edge keys; tiers: L1 w in 0..3 (table1 windows); L2 tier 0 =
    # local bounce (src core == dst core), tiers 1..4 = AllGather chunks.
    tiers = {1: list(range(NW1)), 2: list(range(5))}
    per_core = {1: [], 2: []}
    for k in range(NCORES):
        m = core_e == k
        s, d, nm = ps[m], pd[m], norm_e[m]
        r = d - k * SHARD
        q = r // QUAD
        b = (r % QUAD) // 128
        slot = r % 128
        # layer 1: per-core table1 is rotated so own shard is rows 0..SHARD
        # (spreads self-loop rows uniformly across windows)
        srot = (s - k * SHARD) % NPAD
        w1 = srot // WIN
        g1 = srot - w1 * WIN
        # layer 2
        kc = s // SHARD
        rs = s - kc * SHARD
        local = kc == k
        w2 = np.where(local, 0, 1 + rs // AGCH)
        g2 = np.where(local, rs, kc * AGCH + rs % AGCH)
        per_core[1].append((q, b, w1, g1, nm, slot))
        per_core[2].append((q, b, w2, g2, nm, slot))

    layers = {}
    for L in (1, 2):
        tier_order = tiers[L]
        ntier = len(tier_order)
        # group key in schedule order: (sq, tier_pos, q, b)
        counts = np.zeros((NCORES, NSQ * ntier * SQ * 4), np.int64)

        def gkey(q, b, w):
            # tier position == w for both layers (L1: windows 0..3 in order;
            # L2: 0=local bounce first, then chunks 1..4)
            sq = q // SQ
            return ((sq * ntier + w) * SQ + (q % SQ)) * 4 + b

        keyed = []
        for k in range(NCORES):
            q, b, w, g, nm, slot = per_core[L][k]
            key = gkey(q, b, w)
            order = np.lexsort((g, key))
            keyed.append((key[order], g[order], nm[order], slot[order]))
            counts[k] = np.bincount(key[order], minlength=counts.shape[1])

        T = (np.max(counts, axis=0) + 127) // 128  # tiles per group (shared)
        ngroups = counts.shape[1]
        tile_base = np.zeros(ngroups + 1, np.int64)
        tile_base[1:] = np.cumsum(T)
        t_total = int(tile_base[-1])

        # group metadata in schedule order
        g_sq = np.arange(ngroups) // (ntier * SQ * 4)
        g_tp = (np.arange(ngroups) // (SQ * 4)) % ntier
        g_q = g_sq * SQ + (np.arange(ngroups) // 4) % SQ
        g_b = np.arange(ngroups) % 4
        g_w = np.array([tier_order[tp] if L == 1 else tp for tp in g_tp])

        # per-quad first/last tile: ONE accumulation group per PSUM bank.
        # (HW: a start=True matmul in an open group zeroes the whole bank, so
        # per-(q,b) groups interleaved in one bank wipe each other.)
        qb_seen = set()
        q_first = {}
        q_last = {}
        for g in range(ngroups):
            if T[g] == 0:
                continue
            qb_seen.add((int(g_q[g]), int(g_b[g])))
            q = int(g_q[g])
            if q not in q_first:
                q_first[q] = int(tile_base[g])
            q_last[q] = int(tile_base[g + 1]) - 1
        for q in range(NQ):
            for b in range(4):
                assert (q, b) in qb_seen, f"L{L} empty (q,b)=({q},{b})"

        # tiles metadata: (q, b, start, stop, qalloc)
        tiles_meta = []
        for g in range(ngroups):
            for t in range(int(tile_base[g]), int(tile_base[g + 1])):
                q, b = int(g_q[g]), int(g_b[g])
                tiles_meta.append((
                    q, b,
                    t == q_first[q], t == q_last[q],
                    t == q_first[q],
                ))

        # gather calls: runs of <= CHUNK_T consecutive tiles with same (sq, w)
        calls = []
        g = 0
        while g < ngroups:
            sqw = (int(g_sq[g]), int(g_w[g]))
            ge = g
            while ge < ngroups and (int(g_sq[ge]), int(g_w[ge])) == sqw:
                ge += 1
            t0, tend = int(tile_base[g]), int(tile_base[ge])
            t = t0
            while t < tend:
                nt = min(CHUNK_T, tend - t)
                calls.append((sqw[1], t, nt))
                t += nt
            g = ge
        # super-quad boundaries in call order (for epilogue/dense2 placement)
        sq_call_end = [0] * NSQ
        ci = 0
        g = 0
        for ci, (w, t0, nt) in enumerate(calls):
            # find sq of this call via tile_base
            gi = np.searchsorted(tile_base, t0, side="right") - 1
            sq_call_end[int(g_sq[gi])] = ci + 1

        # per-core flat data
        data = []
        for k in range(NCORES):
            key, g, nm, slot = keyed[k]
            ne = key.shape[0]
            grp_off = np.zeros(ngroups + 1, np.int64)
            grp_off[1:] = np.cumsum(counts[k])
            rank = np.arange(ne, dtype=np.int64) - grp_off[key]
            posn = tile_base[key] * 128 + rank

            gidx = np.zeros(t_total * 128, np.int16)
            nrm = np.zeros(t_total * 128, np.float32)
            dslot = np.zeros(t_total * 128, np.float32)
            gidx[posn] = g.astype(np.int16)
            nrm[posn] = nm
            dslot[posn] = slot.astype(np.float32)

            idxw = np.zeros((128, t_total * 8), np.int16)
            for (w, t0, nt) in calls:
                idxw[:, t0 * 8:(t0 + nt) * 8] = _pack_idx(gidx, t0, nt)
            dstf = dslot.reshape(t_total, 128).T.copy()   # [128, t_total]
            nrmf = nrm.reshape(t_total, 128).T.copy()
            data.append((idxw, dstf, nrmf))

        layers[L] = dict(T=T, t_total=t_total, tiles_meta=tiles_meta,
                         calls=calls, sq_call_end=sq_call_end, data=data)

    # xT input, node-interleaved within each 512 group so dense lhsT slices
    # are contiguous and table writes get 1KB descriptors:
    # column g*512 + m*128 + p holds node g*512 + 4p + m.
    x_pad = np.zeros((NPAD, D), np.float32)
    x_pad[:n] = x
    x_perm = x_pad[node_at_pos]
    bf16 = ml_dtypes.bfloat16

    in_maps = []
    for k in range(NCORES):
        # per-core rotation: table1 row t holds position (t + k*SHARD) % NPAD
        x_rot = np.roll(x_perm, -k * SHARD, axis=0)
        x_il = x_rot.reshape(NPAD // QUAD, 128, 4, D).transpose(0, 2, 1, 3).reshape(NPAD, D)
        xT = np.ascontiguousarray(x_il.T).astype(bf16)
        idx1, dst1, nrm1 = layers[1]["data"][k]
        idx2, dst2, nrm2 = layers[2]["data"][k]
        in_maps.append({
            "xT": xT,
            "W1": W1.astype(bf16),
            "W2": W2.astype(bf16),
            "b1": b1.reshape(128, 1).copy(),
            "b2": b2.reshape(128, 1).copy(),
            "idx1": idx1, "dst1": dst1, "nrm1": nrm1,
            "idx2": idx2, "dst2": dst2, "nrm2": nrm2,
        })

    sched = {L: dict(t_total=layers[L]["t_total"],
                     tiles_meta=tuple(layers[L]["tiles_meta"]),
                     calls=tuple(layers[L]["calls"]),
                     sq_call_end=tuple(layers[L]["sq_call_end"]))
             for L in (1, 2)}
    return in_maps, sched, pos_of_node


def _build_program(sched):
    nc = bacc.Bacc("TRN2", target_bir_lowering=False, debug=False,
                   num_devices=NCORES, num_swdge_queues=4)
    f32 = mybir.dt.float32
    bf16 = mybir.dt.bfloat16
    i16 = mybir.dt.int16
    T1 = sched[1]["t_total"]
    T2 = sched[2]["t_total"]

    xT_d = nc.dram_tensor("xT", [D, NPAD], bf16, kind="ExternalInput")
    W1_d = nc.dram_tensor("W1", [D, D], bf16, kind="ExternalInput")
    W2_d = nc.dram_tensor("W2", [D, D], bf16, kind="ExternalInput")
    b1_d = nc.dram_tensor("b1", [128, 1], f32, kind="ExternalInput")
    b2_d = nc.dram_tensor("b2", [128, 1], f32, kind="ExternalInput")
    idx1_d = nc.dram_tensor("idx1", [128, T1 * 8], i16, kind="ExternalInput")
    dst1_d = nc.dram_tensor("dst1", [128, T1], f32, kind="ExternalInput")
    nrm1_d = nc.dram_tensor("nrm1", [128, T1], f32, kind="ExternalInput")
    idx2_d = nc.dram_tensor("idx2", [128, T2 * 8], i16, kind="ExternalInput")
    dst2_d = nc.dram_tensor("dst2", [128, T2], f32, kind="ExternalInput")
    nrm2_d = nc.dram_tensor("nrm2", [128, T2], f32, kind="ExternalInput")
    out_d = nc.dram_tensor("out", [D, SHARD], f32, kind="ExternalOutput")

    table1 = nc.dram_tensor("table1", [NPAD, D], bf16,
                            kind="ExternalOutput" if K_DEBUG else "Internal")
    bounce2 = nc.dram_tensor("bounce2", [SHARD, D], bf16)
    if K_DEBUG:
        accdump = nc.dram_tensor("accdump", [128, SHARD], bf16, kind="ExternalOutput")
        b2dump = nc.dram_tensor("b2dump", [SHARD, D], bf16, kind="ExternalOutput")
    table2 = [nc.dram_tensor(f"table2_{c}", [NCORES * AGCH, D], bf16,
                             addr_space="Shared") for c in range(4)]

    with tile.TileContext(nc) as tc:
        with (
            tc.tile_pool(name="const", bufs=1) as p_const,
            tc.tile_pool(name="acc", bufs=1) as p_acc,
            tc.tile_pool(name="stage", bufs=4) as p_stage,
            tc.tile_pool(name="msg", bufs=8) as p_msg,
            tc.tile_pool(name="sel", bufs=12) as p_sel,
            tc.tile_pool(name="ost", bufs=3) as p_ost,
            tc.tile_pool(name="dpsum", bufs=2, space="PSUM") as p_dpsum,
            tc.tile_pool(name="qpsum", bufs=6, space="PSUM") as p_qpsum,
        ):
            W1_t = p_const.tile([D, D], bf16)
            W2_t = p_const.tile([D, D], bf16)
            b1_t = p_const.tile([128, 1], f32)
            b2_t = p_const.tile([128, 1], f32)
            iota_t = p_const.tile([128, 128], bf16)
            idx1_t = p_const.tile([128, T1 * 8], i16)
            dst1_t = p_const.tile([128, T1], f32)
            nrm1_t = p_const.tile([128, T1], f32)
            nc.sync.dma_start(out=W1_t[:], in_=W1_d[:])
            nc.sync.dma_start(out=W2_t[:], in_=W2_d[:])
            nc.sync.dma_start(out=b1_t[:], in_=b1_d[:])
            nc.sync.dma_start(out=b2_t[:], in_=b2_d[:])
            nc.sync.dma_start(out=idx1_t[:], in_=idx1_d[:])
            nc.sync.dma_start(out=dst1_t[:], in_=dst1_d[:])
            nc.sync.dma_start(out=nrm1_t[:], in_=nrm1_d[:])
            nc.gpsimd.iota(out=iota_t[:], pattern=[[1, 128]], base=0,
                           channel_multiplier=0,
                           allow_small_or_imprecise_dtypes=True)

            acc1 = p_acc.tile([128, SHARD], bf16)

            # ---------- dense 1 (replicated): table1 = x @ W1, bf16 ----------
            def dense_group(lhs_src, W_t, dram, g):
                """One 512-node group: 4 matmuls -> copy bf16 -> 1KB-desc DMA."""
                ps = p_dpsum.tile([128, QUAD], f32, space="PSUM", tag="dps")
                for m in range(4):
                    nc.tensor.matmul(out=ps[:, m * 128:(m + 1) * 128],
                                     lhsT=lhs_src(m),
                                     rhs=W_t[:], start=True, stop=True)
                st = p_stage.tile([128, QUAD], bf16, tag="stage")
                if g % 2 == 0:
                    nc.scalar.activation(out=st[:], in_=ps[:],
                                         func=mybir.ActivationFunctionType.Copy)
                else:
                    nc.vector.tensor_copy(out=st[:], in_=ps[:])
                dram_ap = dram[g * QUAD:(g + 1) * QUAD, :].rearrange(
                    "(p i) f -> p (i f)", p=128, i=4)
                nc.sync.dma_start(out=dram_ap, in_=st[:])

            with tc.tile_pool(name="xs", bufs=2) as p_x:
                for wi in range(NW1):
                    xc = p_x.tile([128, WIN], bf16, tag="xc")
                    nc.sync.dma_start(out=xc[:], in_=xT_d[:, wi * WIN:(wi + 1) * WIN])
                    for gg in range(WIN // QUAD):
                        g = wi * (WIN // QUAD) + gg
                        dense_group(
                            lambda m, gg=gg, xc=xc: xc[:, gg * QUAD + m * 128: gg * QUAD + (m + 1) * 128],
                            W1_t, table1, g)

            # L2 sidecar loads (SBUF freed by x pool; DMA overlaps L1 edges)
            idx2_t = p_const.tile([128, T2 * 8], i16)
            dst2_t = p_const.tile([128, T2], f32)
            nrm2_t = p_const.tile([128, T2], f32)
            nc.sync.dma_start(out=idx2_t[:], in_=idx2_d[:])
            nc.sync.dma_start(out=dst2_t[:], in_=dst2_d[:])
            nc.sync.dma_start(out=nrm2_t[:], in_=nrm2_d[:])

            # ---------- edge phases ----------
            def edge_phase(L, idx_t, dst_t, nrm_t, src_map, epilogue, post_sq):
                s = sched[L]
                tiles_meta = s["tiles_meta"]
                calls = s["calls"]
                sq_call_end = list(s["sq_call_end"])
                qp = {}
                sq_done = 0
                for ci, (w, t0, nt) in enumerate(calls):
                    msg = p_msg.tile([128, CHUNK_T, 128], bf16, tag="msg")
                    nc.gpsimd.dma_gather(
                        out_ap=msg[:, :nt, :], in_ap=src_map[w],
                        idxs_ap=idx_t[:, t0 * 8:(t0 + nt) * 8],
                        num_idxs=nt * 128, num_idxs_reg=nt * 128,
                        elem_size=D, queue_num=ci % 4)
                    for t in range(t0, t0 + nt):
                        q, b, first, last, qalloc = tiles_meta[t]
                        if qalloc:
                            qp[q] = p_qpsum.tile([128, QUAD], f32, space="PSUM",
                                                 name=f"qp{L}_{q}", tag="qp")
                        S = p_sel.tile([128, 128], bf16, tag="sel")
                        nc.vector.tensor_scalar(
                            S[:], iota_t[:], dst_t[:, t:t + 1], nrm_t[:, t:t + 1],
                            mybir.AluOpType.is_equal, mybir.AluOpType.mult)
                        nc.tensor.matmul(out=qp[q][:, b * 128:(b + 1) * 128],
                                         lhsT=msg[:, t - t0, :], rhs=S[:],
                                         start=first, stop=last,
                                         skip_group_check=True)
                    # super-quad boundary?
                    while sq_done < NSQ and sq_call_end[sq_done] == ci + 1:
                        for q in range(sq_done * SQ, (sq_done + 1) * SQ):
                            epilogue(q, qp.pop(q))
                        post_sq(sq_done)
                        sq_done += 1

            # layer 1: epilogue -> acc1 (interleaved cols), then dense-2 + AG
            def epi1(q, ps):
                # acc1 col q*512 + m*128 + j holds dst slot 4j+m (strided read)
                for m in range(4):
                    nc.scalar.activation(
                        out=acc1[:, q * QUAD + m * 128: q * QUAD + (m + 1) * 128],
                        in_=ps[:, m:QUAD:4],
                        func=mybir.ActivationFunctionType.Relu,
                        bias=b1_t[:, 0:1])

            def post_sq1(sq):
                for qq in range(SQ):
                    g = sq * SQ + qq
                    dense_group(
                        lambda m, g=g: acc1[:, g * QUAD + m * 128: g * QUAD + (m + 1) * 128],
                        W2_t, bounce2, g)
                if sq >= 1:
                    c = sq - 1
                    nc.gpsimd.collective_compute(
                        "AllGather", mybir.AluOpType.bypass,
                        replica_groups=[list(range(NCORES))],
                        ins=[bounce2[c * AGCH:(c + 1) * AGCH, :]],
                        outs=[table2[c][:]],
                    )

            src1 = {w: table1[w * WIN:(w + 1) * WIN, :] for w in range(NW1)}
            edge_phase(1, idx1_t, dst1_t, nrm1_t, src1, epi1, post_sq1)

            if K_DEBUG:
                nc.sync.dma_start(out=accdump[:], in_=acc1[:])
                nc.sync.dma_start(out=b2dump[:], in_=bounce2[:])

            # layer 2: epilogue -> relu -> out_d (transposed; host fixes)
            def epi2(q, ps):
                ost = p_ost.tile([128, QUAD], f32, tag="ost")
                nc.scalar.activation(out=ost[:], in_=ps[:],
                                     func=mybir.ActivationFunctionType.Relu,
                                     bias=b2_t[:, 0:1])
                nc.sync.dma_start(out=out_d[:, q * QUAD:(q + 1) * QUAD], in_=ost[:])

            src2 = {0: bounce2[:]}
            for c in range(4):
                src2[1 + c] = table2[c][:]
            edge_phase(2, idx2_t, dst2_t, nrm2_t, src2, epi2,
                       lambda sq: None)

    nc.compile()
    return nc


def prepare(x, edge_index, W1, b1, W2, b2):
    in_maps, sched, pos_of_node = _host_prep(x, edge_index, W1, b1, W2, b2)
    key = (sched[1]["tiles_meta"], sched[1]["calls"],
           sched[2]["tiles_meta"], sched[2]["calls"])
    if key not in _CACHE:
        _CACHE[key] = _build_program(sched)
    return _CACHE[key], in_maps, pos_of_node


def kernel(x, edge_index, W1, b1, W2, b2):
    nc, in_maps, pos_of_node = prepare(x, edge_index, W1, b1, W2, b2)
    res = run_bass_kernel_spmd(nc, in_maps, list(range(NCORES)))
    full = np.concatenate(
        [np.asarray(res.results[k]["out"], dtype=np.float32).T for k in range(NCORES)],
        axis=0)
    n = np.asarray(x).shape[0]
    return full[pos_of_node[:n]]
